# revision 6
# baseline (speedup 1.0000x reference)
"""Trainium2 Bass kernel for nn_AbsWordCopySummarizer (8 NeuronCores).

out = p * softmax(dec @ W_gen^T + b_gen) + (1-p) * scatter_v(mean_h(attn), words)

Sharding strategy (SPMD, one NEFF on 8 cores; per-core behavior rides on data):
  * Vocab-parallel: core c owns vocab columns [c*3840, (c+1)*3840) of W_gen^T
    and of the output. The generator matmul (fp32r on the PE), the softmax
    exp, and the final gated mix run per-shard.
  * Phase A is batch-parallel: core b processes attention for batch b only
    (head-mean, gate p[b], duplicate-combined copy weights cw^T*(1-p)), then
    one AllGather shares (cw, p) with all cores.
  * Cross-shard softmax: each core computes a per-row local max m_c (max of
    its first 480-wide tile) and the local sum s_c of exp(l - m_c); a second
    8KB AllGather shares (m_c, s_c); every core then forms the global max M,
    S = sum_c s_c*exp(m_c - M), and folds p * exp(m_own - M) / S into one
    per-row scale applied in the final mix.
  * The scatter-add becomes a dense one-hot matmul per (batch, vocab tile):
    one-hot built on device via iota + is_equal against host-deduplicated
    shard-local indices; duplicate words are pre-combined through a host
    0/1-matrix C applied as a matmul, so no read-modify-write is needed.

Host work is layout/index-only: transpose + shard W_gen, transpose dec,
build index/combining matrices from `words`, concatenate output shards.
"""
import contextlib
import ctypes
import sys
import types

import numpy as np

# ---------------------------------------------------------------------------
# Environment shims (self-contained).
# ---------------------------------------------------------------------------


def _install_ntff_shim():
    """Provide antenv.axon_hooks (absent in this container) so profiling via
    BASS_TRACE=1 works when a harness requests it."""
    if "antenv.axon_hooks" in sys.modules:
        return
    try:
        import antenv
    except ImportError:
        return

    state = {"hook": None}

    def _build_hook():
        try:
            lib = ctypes.CDLL("/opt/axon/libaxon_pjrt.so")
        except OSError:
            return None
        if not hasattr(lib, "axon_start_nrt_profile"):
            return None
        lib.axon_start_nrt_profile.argtypes = [
            ctypes.POINTER(ctypes.c_int64),
            ctypes.c_size_t,
        ]
        lib.axon_start_nrt_profile.restype = ctypes.c_int64
        lib.axon_stop_nrt_profile.argtypes = [ctypes.c_char_p]
        lib.axon_stop_nrt_profile.restype = ctypes.c_int64

        @contextlib.contextmanager
        def _hook(output_dir, device_ids):
            import jax

            jax.devices()
            if device_ids:
                ids = (ctypes.c_int64 * len(device_ids))(*device_ids)
                rc = lib.axon_start_nrt_profile(ids, len(device_ids))
            else:
                rc = lib.axon_start_nrt_profile(None, 0)
            if rc != 0:
                raise RuntimeError(f"axon_start_nrt_profile rc={rc}")
            try:
                yield
            finally:
                n = lib.axon_stop_nrt_profile(str(output_dir).encode())
                if n < 0:
                    raise RuntimeError(f"axon_stop_nrt_profile rc={n}")

        return _hook

    mod = types.ModuleType("antenv.axon_hooks")
    mod.get_axon_ntff_profile_hook = lambda: state["hook"]
    mod.set_axon_ntff_profile_hook = lambda h: state.update(hook=h)
    sys.modules["antenv.axon_hooks"] = mod
    antenv.axon_hooks = mod
    state["hook"] = _build_hook()


_install_ntff_shim()

import concourse.bass as bass  # noqa: E402
import concourse.mybir as mybir  # noqa: E402
import concourse.tile as tile  # noqa: E402
from concourse.bass_utils import run_bass_kernel_spmd  # noqa: E402
from concourse.masks import make_identity  # noqa: E402
from concourse.vector_clock import ScopedClock  # noqa: E402


def _patched_drain_and_barrier(self, tick_clock, wait_clock):
    # This container's walrus rejects >1 sync-wait per instruction; split the
    # kernel-tail drain's waits across single-wait nops.
    nc = self.nc
    carrier = nc.sync.nop()
    wait_clock.add_sem_waits(
        carrier.ins, ScopedClock({None: tick_clock.global_clock})
    )
    waits = list(carrier.ins.sync_info.on_wait)
    carrier.ins.sync_info.on_wait = waits[:1]
    for w in waits[1:]:
        n = nc.sync.nop()
        n.ins.sync_info = mybir.SyncInfo(on_wait=[w], on_update=[])
    nc.sync.drain()
    nc.all_engine_barrier()
    assert self.sems is not None
    popped = nc._tile_sem_poison_stack.pop()
    assert popped is self._sem_poison
    nc.clear_and_free_semaphores(list(self.sems.allocated().values()))
    nc.all_engine_barrier()


tile.TileContext._drain_and_barrier = _patched_drain_and_barrier


def _fix_multiwait(nc, max_waits: int = 1):
    """Move excess per-instruction sync-waits onto same-engine NoOps (this
    walrus build caps waits-per-instruction)."""
    import bass_rust

    ctr = [0]

    def mknop(engine, waits):
        ctr[0] += 1
        n = bass_rust.InstNoOp(name=f"waitfix-{ctr[0]}", ins=[], outs=[])
        n.engine = engine
        n.sync_info = mybir.SyncInfo(on_wait=list(waits), on_update=[])
        return n

    for fn in nc.m.functions:
        for bb in fn.blocks:
            out = []
            changed = False
            for ins in bb.instructions:
                si = ins.sync_info
                waits = list(si.on_wait) if si is not None else []
                if len(waits) > max_waits:
                    changed = True
                    extra = waits[: len(waits) - max_waits]
                    keep = waits[len(waits) - max_waits:]
                    for i in range(0, len(extra), max_waits):
                        out.append(mknop(ins.engine, extra[i:i + max_waits]))
                    ins.sync_info = mybir.SyncInfo(
                        on_wait=keep, on_update=list(si.on_update)
                    )
                out.append(ins)
            if changed:
                bb.instructions = out


# ---------------------------------------------------------------------------
# Problem constants.
# ---------------------------------------------------------------------------
B, HEADS, T, S, H = 8, 8, 128, 512, 768
V = 30522
CORES = 8
VSH = 3840           # vocab columns per core; 8*3840 = 30720 >= 30522
NT = 480             # psum tile width
NVT = VSH // NT      # 8
KC = H // 128        # 6
SC = S // 128        # 4
KU = 128             # padded unique-words per (core, batch)

F32 = mybir.dt.float32
F32R = mybir.dt.float32r
BF16 = mybir.dt.bfloat16
EXP_DT = BF16        # resident exp buffer dtype
NEG_BIG = -1.0e30

AX = mybir.AluOpType
AF = mybir.ActivationFunctionType

CWN = SC * KU        # not used; kept for clarity
AGW = CORES * KU + 8  # AllGather#1 row width: 8 shard blocks + p + pad

_CACHE = {}


def _gen_kernel():
    nc = bass.Bass(target_bir_lowering=False)

    attn_b = nc.dram_tensor("attn_b", [HEADS, T, S], F32, kind="ExternalInput")
    wv_b = nc.dram_tensor("wv_b", [S, H], F32, kind="ExternalInput")
    dec_b = nc.dram_tensor("dec_b", [T, H], F32, kind="ExternalInput")
    cmb = nc.dram_tensor("cmb", [S, CORES, KU], F32, kind="ExternalInput")
    wp = nc.dram_tensor("wp", [T, 2 * H], F32, kind="ExternalInput")
    bp = nc.dram_tensor("bp", [T, 1], F32, kind="ExternalInput")
    selv = nc.dram_tensor("selv", [T, CORES], F32, kind="ExternalInput")
    dect = nc.dram_tensor("dect", [B, H, T], F32R, kind="ExternalInput")
    wgt = nc.dram_tensor("wgt", [H, VSH], F32R, kind="ExternalInput")
    bg = nc.dram_tensor("bg", [1, VSH], F32R, kind="ExternalInput")
    ones1 = nc.dram_tensor("ones1", [1, T], F32R, kind="ExternalInput")
    u_loc = nc.dram_tensor("u_loc", [B, KU, 1], F32, kind="ExternalInput")

    outp = nc.dram_tensor("outp", [B, T, VSH], F32, kind="ExternalOutput")

    with tile.TileContext(nc) as tc:
        with (
            tc.tile_pool(name="res", bufs=1) as res,
            tc.tile_pool(name="dram", bufs=1, space="DRAM") as dram,
        ):
            ident = res.tile([128, 128], F32)
            make_identity(nc, ident[:])

            dect_sb = res.tile([128, B * KC * T], F32R)
            nc.sync.dma_start(
                out=dect_sb[:].rearrange("p (b kc m) -> p b kc m", b=B, kc=KC),
                in_=dect.rearrange("b (kc p) m -> p b kc m", p=128),
            )
            dectv = dect_sb[:].rearrange("p (b kc m) -> p b kc m", b=B, kc=KC)
            ones_sb = res.tile([1, T], F32R)
            nc.sync.dma_start(out=ones_sb[:], in_=ones1[:])
            bg_sb = res.tile([1, VSH], F32R)
            nc.sync.dma_start(out=bg_sb[:], in_=bg[:])
            u_sb = res.tile([128, B], F32)
            nc.sync.dma_start(
                out=u_sb[:].rearrange("p (b one) -> p b one", b=B),
                in_=u_loc.rearrange("b p one -> p b one"),
            )
            selv_sb = res.tile([128, CORES], F32)
            nc.sync.dma_start(out=selv_sb[:], in_=selv[:])

            mparts = res.tile([128, B], F32)
            negm = res.tile([128, B], F32)
            ssump = res.tile([128, B * NVT], F32)
            a_all = res.tile([128, B], F32)
            p_sb = res.tile([128, B], F32)
            cwq_sb = [
                res.tile([128, 128], F32R, tag=f"cwq{b}", name=f"cwq{b}")
                for b in range(B)
            ]

            cw_in = dram.tile([128, AGW], F32)
            cw_out = dram.tile([CORES * 128, AGW], F32)
            ms_in = dram.tile([128, 2 * B], F32)
            ms_out = dram.tile([CORES * 128, 2 * B], F32)

            # ================= Phase A (own batch only) ===================
            with (
                tc.tile_pool(name="pa", bufs=1) as pa,
                tc.tile_pool(name="pa_ps", bufs=2, space="PSUM") as pa_ps,
            ):
                att = pa.tile([128, HEADS * S], F32)
                nc.sync.dma_start(
                    out=att[:].rearrange("p (h s) -> p h s", h=HEADS),
                    in_=attn_b.rearrange("h t s -> t h s"),
                )
                attv = att[:].rearrange("p (h s) -> p h s", h=HEADS)
                s01 = pa.tile([128, S], F32)
                s23 = pa.tile([128, S], F32)
                s45 = pa.tile([128, S], F32)
                s67 = pa.tile([128, S], F32)
                nc.vector.tensor_tensor(out=s01[:], in0=attv[:, 0], in1=attv[:, 1], op=AX.add)
                nc.vector.tensor_tensor(out=s23[:], in0=attv[:, 2], in1=attv[:, 3], op=AX.add)
                nc.vector.tensor_tensor(out=s45[:], in0=attv[:, 4], in1=attv[:, 5], op=AX.add)
                nc.vector.tensor_tensor(out=s67[:], in0=attv[:, 6], in1=attv[:, 7], op=AX.add)
                nc.vector.tensor_tensor(out=s01[:], in0=s01[:], in1=s23[:], op=AX.add)
                nc.vector.tensor_tensor(out=s45[:], in0=s45[:], in1=s67[:], op=AX.add)
                aw = pa.tile([128, S], F32)
                nc.vector.tensor_tensor(out=aw[:], in0=s01[:], in1=s45[:], op=AX.add)

                awT = pa.tile([128, S], F32)
                for j in range(SC):
                    tp = pa_ps.tile([128, 128], F32, tag="tp")
                    nc.tensor.transpose(
                        out=tp[:], in_=aw[:, j * 128:(j + 1) * 128], identity=ident[:]
                    )
                    nc.vector.tensor_copy(out=awT[:, j * 128:(j + 1) * 128], in_=tp[:])

                wvt = pa.tile([128, SC * H], F32)
                nc.sync.dma_start(
                    out=wvt[:].rearrange("p (j h) -> p j h", j=SC),
                    in_=wv_b.rearrange("(j p) h -> p j h", p=128),
                )
                wvv = wvt[:].rearrange("p (j h) -> p j h", j=SC)
                wp_sb = pa.tile([128, 2 * H], F32)
                nc.sync.dma_start(out=wp_sb[:], in_=wp[:])
                bp_sb = pa.tile([128, 1], F32)
                nc.sync.dma_start(out=bp_sb[:], in_=bp[:])

                # pre2 = mean_h(attn) @ (word_vec @ Wp2): r[s] on DVE, then
                # 4 accumulated N=1 matmuls with awT as lhsT. (1/8 of the head
                # mean is folded into wp[:, H:] on the host.)
                scr = pa.tile([128, H], F32)
                rvec = pa.tile([128, SC], F32)
                for j in range(SC):
                    nc.vector.tensor_tensor(
                        out=scr[:], in0=wvv[:, j], in1=wp_sb[:, H:2 * H], op=AX.mult
                    )
                    nc.vector.tensor_reduce(
                        out=rvec[:, j:j + 1], in_=scr[:],
                        axis=mybir.AxisListType.X, op=AX.add,
                    )
                pre2 = pa_ps.tile([128, 1], F32, tag="pre2")
                for j in range(SC):
                    nc.tensor.matmul(
                        out=pre2[:], lhsT=awT[:, j * 128:(j + 1) * 128],
                        rhs=rvec[:, j:j + 1], start=(j == 0), stop=(j == SC - 1),
                    )

                dec_sb = pa.tile([128, H], F32)
                nc.sync.dma_start(out=dec_sb[:], in_=dec_b[:])
                scr2 = pa.tile([128, H], F32)
                pre1 = pa.tile([128, 1], F32)
                nc.vector.tensor_tensor(
                    out=scr2[:], in0=dec_sb[:], in1=wp_sb[:, 0:H], op=AX.mult
                )
                nc.vector.tensor_reduce(
                    out=pre1[:], in_=scr2[:], axis=mybir.AxisListType.X, op=AX.add
                )
                nc.vector.tensor_tensor(out=pre1[:], in0=pre1[:], in1=bp_sb[:], op=AX.add)
                pre = pa.tile([128, 1], F32)
                nc.vector.tensor_tensor(out=pre[:], in0=pre1[:], in1=pre2[:], op=AX.add)
                p_t = pa.tile([128, 1], F32)
                nc.scalar.activation(out=p_t[:], in_=pre[:], func=AF.Sigmoid)
                q_t = pa.tile([128, 1], F32)
                nc.vector.tensor_scalar(
                    out=q_t[:], in0=p_t[:], scalar1=-1.0, scalar2=1.0,
                    op0=AX.mult, op1=AX.add,
                )

                awq = pa.tile([128, S], F32)
                nc.vector.tensor_scalar(
                    out=awq[:], in0=aw[:], scalar1=q_t[:, 0:1], scalar2=None,
                    op0=AX.mult,
                )
                awqT = pa.tile([128, S], F32)
                for j in range(SC):
                    tp = pa_ps.tile([128, 128], F32, tag="tp")
                    nc.tensor.transpose(
                        out=tp[:], in_=awq[:, j * 128:(j + 1) * 128], identity=ident[:]
                    )
                    nc.vector.tensor_copy(out=awqT[:, j * 128:(j + 1) * 128], in_=tp[:])

                cmb_sb = pa.tile([128, SC * CORES * KU], F32)
                nc.sync.dma_start(
                    out=cmb_sb[:].rearrange("p (j c k) -> p j c k", j=SC, c=CORES),
                    in_=cmb.rearrange("(j p) c k -> p j c k", p=128),
                )
                cmbv = cmb_sb[:].rearrange("p (j c k) -> p j c k", j=SC, c=CORES)
                contrib = pa.tile([128, AGW], F32)
                for c in range(CORES):
                    cps = pa_ps.tile([128, 128], F32, tag="cps")
                    for j in range(SC):
                        nc.tensor.matmul(
                            out=cps[:], lhsT=cmbv[:, j, c],
                            rhs=awqT[:, j * 128:(j + 1) * 128],
                            start=(j == 0), stop=(j == SC - 1),
                        )
                    nc.vector.tensor_copy(out=contrib[:, c * KU:(c + 1) * KU], in_=cps[:])
                nc.vector.tensor_copy(
                    out=contrib[:, CORES * KU:CORES * KU + 1], in_=p_t[:]
                )
                nc.gpsimd.memset(contrib[:, CORES * KU + 1:AGW], 0.0)
                nc.gpsimd.dma_start(out=cw_in[:], in_=contrib[:])

            nc.gpsimd.collective_compute(
                "AllGather", AX.bypass,
                replica_groups=[list(range(CORES))],
                ins=[cw_in.opt()], outs=[cw_out.opt()],
            )

            # p for all batches: column AGW-8 of each batch-block
            nc.sync.dma_start(
                out=p_sb[:].rearrange("p (b one) -> p b one", b=B),
                in_=cw_out.rearrange("(b p) n -> p b n", p=128)[
                    :, :, CORES * KU:CORES * KU + 1
                ],
            )

            # Own-shard cw selection: cw_own[b] = sum_c selv[c] * block_c.
            with tc.tile_pool(name="psel", bufs=2) as psel:
                for b in range(B):
                    blocks = psel.tile([128, CORES * KU], F32, tag="blocks")
                    nc.sync.dma_start(
                        out=blocks[:],
                        in_=cw_out[b * 128:(b + 1) * 128, 0:CORES * KU],
                    )
                    acc0 = psel.tile([128, KU], F32, tag="acc0")
                    acc1 = psel.tile([128, KU], F32, tag="acc1")
                    nc.vector.tensor_scalar(
                        out=acc0[:], in0=blocks[:, 0:KU],
                        scalar1=selv_sb[:, 0:1], scalar2=None, op0=AX.mult,
                    )
                    cur, nxt = acc0, acc1
                    for c in range(1, CORES):
                        nc.vector.scalar_tensor_tensor(
                            out=nxt[:], in0=blocks[:, c * KU:(c + 1) * KU],
                            scalar=selv_sb[:, c:c + 1], in1=cur[:],
                            op0=AX.mult, op1=AX.add,
                        )
                        cur, nxt = nxt, cur
                    nc.vector.tensor_copy(out=cwq_sb[b][:], in_=cur[:])

            # ================= Phase G: generator matmul + exp ============
            with tc.tile_pool(name="pexp", bufs=1) as pexp:
                exp_t = [
                    pexp.tile([128, VSH], EXP_DT, tag=f"exp{b}", name=f"exp{b}")
                    for b in range(B)
                ]
                with (
                    tc.tile_pool(name="pg", bufs=2) as pg,
                    tc.tile_pool(name="pg_ps", bufs=6, space="PSUM") as pg_ps,
                ):
                    for vt in range(NVT):
                        wgt_t = pg.tile([128, KC * NT], F32R, tag="wgt_t")
                        nc.sync.dma_start(
                            out=wgt_t[:].rearrange("p (kc n) -> p kc n", kc=KC),
                            in_=wgt.rearrange("(kc p) v -> p kc v", p=128)[
                                :, :, vt * NT:(vt + 1) * NT
                            ],
                        )
                        wgv = wgt_t[:].rearrange("p (kc n) -> p kc n", kc=KC)
                        for b in range(B):
                            ps = pg_ps.tile([128, NT], F32, tag="gen")
                            for kc in range(KC):
                                nc.tensor.matmul(
                                    out=ps[:], lhsT=dectv[:, b, kc], rhs=wgv[:, kc],
                                    start=(kc == 0), stop=False,
                                )
                            nc.tensor.matmul(
                                out=ps[:], lhsT=ones_sb[0:1, :],
                                rhs=bg_sb[0:1, vt * NT:(vt + 1) * NT],
                                start=False, stop=True,
                            )
                            if vt == 0:
                                nc.vector.tensor_reduce(
                                    out=mparts[:, b:b + 1], in_=ps[:],
                                    axis=mybir.AxisListType.X, op=AX.max,
                                )
                                nc.vector.tensor_scalar(
                                    out=negm[:, b:b + 1], in0=mparts[:, b:b + 1],
                                    scalar1=-1.0, scalar2=None, op0=AX.mult,
                                )
                            nc.scalar.activation(
                                out=exp_t[b][:, vt * NT:(vt + 1) * NT], in_=ps[:],
                                func=AF.Exp, bias=negm[:, b:b + 1], scale=1.0,
                                accum_out=ssump[:, b * NVT + vt:b * NVT + vt + 1],
                            )

                # local sums -> AllGather#2 of (m_c, s_c)
                with tc.tile_pool(name="pms", bufs=1) as pms:
                    msc = pms.tile([128, 2 * B], F32)
                    nc.vector.tensor_copy(out=msc[:, 0:B], in_=mparts[:])
                    ssv = ssump[:].rearrange("p (b v) -> p b v", b=B)
                    for b in range(B):
                        nc.vector.tensor_reduce(
                            out=msc[:, B + b:B + b + 1], in_=ssv[:, b],
                            axis=mybir.AxisListType.X, op=AX.add,
                        )
                    nc.gpsimd.dma_start(out=ms_in[:], in_=msc[:])

                nc.gpsimd.collective_compute(
                    "AllGather", AX.bypass,
                    replica_groups=[list(range(CORES))],
                    ins=[ms_in.opt()], outs=[ms_out.opt()],
                )

                # global (M, S) and the per-row scale a = p*exp(m_own-M)/S
                with tc.tile_pool(name="pfx", bufs=1) as pfx:
                    agg = pfx.tile([128, CORES * 2 * B], F32)
                    nc.sync.dma_start(
                        out=agg[:].rearrange("p (c n) -> p c n", c=CORES),
                        in_=ms_out.rearrange("(c p) n -> p c n", p=128),
                    )
                    aggv = agg[:].rearrange("p (c n) -> p c n", c=CORES)
                    em = pfx.tile([128, CORES], F32)
                    junk = pfx.tile([128, CORES], F32)
                    bigm = pfx.tile([128, 1], F32)
                    nbigm = pfx.tile([128, 1], F32)
                    ssum = pfx.tile([128, 1], F32)
                    sinv = pfx.tile([128, 1], F32)
                    eo = pfx.tile([128, 1], F32)
                    for b in range(B):
                        nc.vector.tensor_reduce(
                            out=bigm[:], in_=aggv[:, :, b],
                            axis=mybir.AxisListType.X, op=AX.max,
                        )
                        nc.vector.tensor_scalar(
                            out=nbigm[:], in0=bigm[:], scalar1=-1.0,
                            scalar2=None, op0=AX.mult,
                        )
                        nc.scalar.activation(
                            out=em[:], in_=aggv[:, :, b], func=AF.Exp,
                            bias=nbigm[:, 0:1], scale=1.0,
                        )
                        nc.vector.tensor_tensor(
                            out=junk[:], in0=em[:], in1=aggv[:, :, B + b], op=AX.mult
                        )
                        nc.vector.tensor_reduce(
                            out=ssum[:], in_=junk[:],
                            axis=mybir.AxisListType.X, op=AX.add,
                        )
                        nc.vector.reciprocal(out=sinv[:], in_=ssum[:])
                        # exp(m_own - M)
                        nc.vector.tensor_tensor(
                            out=eo[:], in0=mparts[:, b:b + 1], in1=bigm[:],
                            op=AX.subtract,
                        )
                        nc.scalar.activation(out=eo[:], in_=eo[:], func=AF.Exp)
                        nc.vector.tensor_tensor(
                            out=eo[:], in0=eo[:], in1=sinv[:], op=AX.mult
                        )
                        nc.vector.tensor_tensor(
                            out=a_all[:, b:b + 1], in0=eo[:], in1=p_sb[:, b:b + 1],
                            op=AX.mult,
                        )

                # ================= Phase M: one-hot corr + mix + store ====
                with (
                    tc.tile_pool(name="pm", bufs=3) as pm,
                    tc.tile_pool(name="pm_ps", bufs=4, space="PSUM") as pm_ps,
                ):
                    for vt in range(NVT):
                        iot = pm.tile([128, NT], F32, tag="iot")
                        nc.gpsimd.iota(
                            iot[:], pattern=[[1, NT]], base=vt * NT,
                            channel_multiplier=0,
                            allow_small_or_imprecise_dtypes=True,
                        )
                        for b in range(B):
                            oh = pm.tile([128, NT], F32R, tag="oh")
                            nc.vector.tensor_scalar(
                                out=oh[:], in0=iot[:], scalar1=u_sb[:, b:b + 1],
                                scalar2=None, op0=AX.is_equal,
                            )
                            av = pm_ps.tile([128, NT], F32, tag="av")
                            nc.tensor.matmul(
                                out=av[:], lhsT=cwq_sb[b][:], rhs=oh[:],
                                start=True, stop=True,
                            )
                            ot = pm.tile([128, NT], F32, tag="ot")
                            nc.vector.scalar_tensor_tensor(
                                out=ot[:], in0=exp_t[b][:, vt * NT:(vt + 1) * NT],
                                scalar=a_all[:, b:b + 1], in1=av[:],
                                op0=AX.mult, op1=AX.add,
                            )
                            nc.sync.dma_start(
                                out=outp[b, :, vt * NT:(vt + 1) * NT], in_=ot[:]
                            )

    _fix_multiwait(nc)
    return nc


# ---------------------------------------------------------------------------
# Host-side driver.
# ---------------------------------------------------------------------------


def _prep_inputs(decoder_outputs, attn_dist, word_vec, words, W_gen, b_gen, W_p, b_p):
    f32 = np.float32
    dec = np.asarray(decoder_outputs, f32)
    attn = np.asarray(attn_dist, f32)
    wv = np.asarray(word_vec, f32)
    words = np.asarray(words)
    W_gen = np.asarray(W_gen, f32)
    b_gen = np.asarray(b_gen, f32)
    W_p = np.asarray(W_p, f32)
    b_p = np.asarray(b_p, f32)

    WT = np.ascontiguousarray(W_gen.T)                      # [H, V]
    wgt_pad = np.zeros((H, CORES * VSH), f32)
    wgt_pad[:, :V] = WT
    bg_pad = np.full((CORES * VSH,), NEG_BIG, f32)
    bg_pad[:V] = b_gen
    dect = np.ascontiguousarray(dec.transpose(0, 2, 1))     # [B, H, T]

    wp_scaled = np.tile(W_p.reshape(1, 2 * H), (T, 1)).astype(f32)
    wp_scaled[:, H:] *= 1.0 / HEADS                          # fold head-mean
    bp_b = np.full((T, 1), float(b_p.reshape(-1)[0]), f32)
    ones_row = np.ones((1, T), f32)

    # per (core, batch) dedup: local indices + combining matrix
    u_all = np.full((CORES, B, KU, 1), -1.0, f32)
    cmb_all = np.zeros((B, S, CORES, KU), f32)
    for b in range(B):
        w = np.asarray(words[b], np.int64)
        for c in range(CORES):
            lo, hi = c * VSH, (c + 1) * VSH
            mask = (w >= lo) & (w < hi)
            uniq = np.unique(w[mask])
            k = len(uniq)
            assert k <= KU, f"unique words {k} exceeds KU={KU}"
            u_all[c, b, :k, 0] = (uniq - lo).astype(f32)
            if k:
                pos = np.searchsorted(uniq, w[mask])
                cmb_all[b, np.nonzero(mask)[0], c, pos] = 1.0 / HEADS

    in_maps = []
    for c in range(CORES):
        sel = np.zeros((T, CORES), f32)
        sel[:, c] = 1.0
        in_maps.append({
            "attn_b": np.ascontiguousarray(attn[c]),
            "wv_b": np.ascontiguousarray(wv[c]),
            "dec_b": np.ascontiguousarray(dec[c]),
            "cmb": np.ascontiguousarray(cmb_all[c]),
            "wp": wp_scaled,
            "bp": bp_b,
            "selv": sel,
            "dect": dect,
            "wgt": np.ascontiguousarray(wgt_pad[:, c * VSH:(c + 1) * VSH]),
            "bg": np.ascontiguousarray(bg_pad[c * VSH:(c + 1) * VSH]).reshape(1, VSH),
            "ones1": ones_row,
            "u_loc": np.ascontiguousarray(u_all[c]),
        })
    return in_maps


LAST_RESULTS = None


def kernel(decoder_outputs, attn_dist, word_vec, words, W_gen, b_gen, W_p, b_p):
    global LAST_RESULTS
    in_maps = _prep_inputs(
        decoder_outputs, attn_dist, word_vec, words, W_gen, b_gen, W_p, b_p
    )
    if "nc" not in _CACHE:
        _CACHE["nc"] = _gen_kernel()
    nc = _CACHE["nc"]
    res = run_bass_kernel_spmd(nc, in_maps, core_ids=list(range(CORES)))
    LAST_RESULTS = res
    out = np.empty((B, T, V), np.float32)
    for c in range(CORES):
        lo = c * VSH
        hi = min(V, lo + VSH)
        out[:, :, lo:hi] = res.results[c]["outp"][:, :, : hi - lo]
    return out


# revision 28
# speedup vs baseline: 1.9228x; 1.9228x over previous
"""Trainium2 Bass kernel for nn_AbsWordCopySummarizer (8 NeuronCores).

out = p * softmax(dec @ W_gen^T + b_gen) + (1-p) * scatter_v(mean_h(attn), words)

Sharding strategy (SPMD, one NEFF on 8 cores; per-core behavior rides on data):
  * Vocab-parallel: core c owns vocab columns [c*3840, (c+1)*3840) of W_gen^T
    and of the output. The generator matmul (fp32r on the PE), the softmax
    exp, and the final gated mix run per-shard.
  * Phase A is batch-parallel: core b processes attention for batch b only
    (head-mean, gate p[b], duplicate-combined copy weights cw^T*(1-p)), then
    one AllGather shares (cw, p) with all cores.
  * Cross-shard softmax: each core computes a per-row local max m_c (max of
    its first 480-wide tile) and the local sum s_c of exp(l - m_c); a second
    8KB AllGather shares (m_c, s_c); every core then forms the global max M,
    S = sum_c s_c*exp(m_c - M), and folds p * exp(m_own - M) / S into one
    per-row scale applied in the final mix.
  * The scatter-add becomes a dense one-hot matmul per (batch, vocab tile):
    one-hot built on device via iota + is_equal against host-deduplicated
    shard-local indices; duplicate words are pre-combined through a host
    0/1-matrix C applied as a matmul, so no read-modify-write is needed.

Host work is layout/index-only: transpose + shard W_gen, transpose dec,
build index/combining matrices from `words`, concatenate output shards.
"""
import contextlib
import ctypes
import sys
import types

import numpy as np
import ml_dtypes

BF16_NP = ml_dtypes.bfloat16

# ---------------------------------------------------------------------------
# Environment shims (self-contained).
# ---------------------------------------------------------------------------


def _install_ntff_shim():
    """Provide antenv.axon_hooks (absent in this container) so profiling via
    BASS_TRACE=1 works when a harness requests it."""
    if "antenv.axon_hooks" in sys.modules:
        return
    try:
        import antenv
    except ImportError:
        return

    state = {"hook": None}

    def _build_hook():
        try:
            lib = ctypes.CDLL("/opt/axon/libaxon_pjrt.so")
        except OSError:
            return None
        if not hasattr(lib, "axon_start_nrt_profile"):
            return None
        lib.axon_start_nrt_profile.argtypes = [
            ctypes.POINTER(ctypes.c_int64),
            ctypes.c_size_t,
        ]
        lib.axon_start_nrt_profile.restype = ctypes.c_int64
        lib.axon_stop_nrt_profile.argtypes = [ctypes.c_char_p]
        lib.axon_stop_nrt_profile.restype = ctypes.c_int64

        @contextlib.contextmanager
        def _hook(output_dir, device_ids):
            import jax

            jax.devices()
            if device_ids:
                ids = (ctypes.c_int64 * len(device_ids))(*device_ids)
                rc = lib.axon_start_nrt_profile(ids, len(device_ids))
            else:
                rc = lib.axon_start_nrt_profile(None, 0)
            if rc != 0:
                raise RuntimeError(f"axon_start_nrt_profile rc={rc}")
            try:
                yield
            finally:
                n = lib.axon_stop_nrt_profile(str(output_dir).encode())
                if n < 0:
                    raise RuntimeError(f"axon_stop_nrt_profile rc={n}")

        return _hook

    mod = types.ModuleType("antenv.axon_hooks")
    mod.get_axon_ntff_profile_hook = lambda: state["hook"]
    mod.set_axon_ntff_profile_hook = lambda h: state.update(hook=h)
    sys.modules["antenv.axon_hooks"] = mod
    antenv.axon_hooks = mod
    state["hook"] = _build_hook()


_install_ntff_shim()

import concourse.bass as bass  # noqa: E402
import concourse.mybir as mybir  # noqa: E402
import concourse.tile as tile  # noqa: E402
from concourse.bass_utils import run_bass_kernel_spmd  # noqa: E402
from concourse.masks import make_identity  # noqa: E402
from concourse.vector_clock import ScopedClock  # noqa: E402


def _patched_drain_and_barrier(self, tick_clock, wait_clock):
    # This container's walrus rejects >1 sync-wait per instruction; split the
    # kernel-tail drain's waits across single-wait nops.
    nc = self.nc
    carrier = nc.sync.nop()
    wait_clock.add_sem_waits(
        carrier.ins, ScopedClock({None: tick_clock.global_clock})
    )
    waits = list(carrier.ins.sync_info.on_wait)
    carrier.ins.sync_info.on_wait = waits[:1]
    for w in waits[1:]:
        n = nc.sync.nop()
        n.ins.sync_info = mybir.SyncInfo(on_wait=[w], on_update=[])
    nc.sync.drain()
    nc.all_engine_barrier()
    assert self.sems is not None
    popped = nc._tile_sem_poison_stack.pop()
    assert popped is self._sem_poison
    nc.clear_and_free_semaphores(list(self.sems.allocated().values()))
    nc.all_engine_barrier()


tile.TileContext._drain_and_barrier = _patched_drain_and_barrier


def _fix_multiwait(nc, max_waits: int = 1):
    """Move excess per-instruction sync-waits onto same-engine NoOps (this
    walrus build caps waits-per-instruction)."""
    import bass_rust

    ctr = [0]

    def mknop(engine, waits):
        ctr[0] += 1
        n = bass_rust.InstNoOp(name=f"waitfix-{ctr[0]}", ins=[], outs=[])
        n.engine = engine
        n.sync_info = mybir.SyncInfo(on_wait=list(waits), on_update=[])
        return n

    for fn in nc.m.functions:
        for bb in fn.blocks:
            out = []
            changed = False
            for ins in bb.instructions:
                si = ins.sync_info
                waits = list(si.on_wait) if si is not None else []
                if len(waits) > max_waits:
                    changed = True
                    extra = waits[: len(waits) - max_waits]
                    keep = waits[len(waits) - max_waits:]
                    for i in range(0, len(extra), max_waits):
                        out.append(mknop(ins.engine, extra[i:i + max_waits]))
                    ins.sync_info = mybir.SyncInfo(
                        on_wait=keep, on_update=list(si.on_update)
                    )
                out.append(ins)
            if changed:
                bb.instructions = out


# ---------------------------------------------------------------------------
# Problem constants.
# ---------------------------------------------------------------------------
B, HEADS, T, S, H = 8, 8, 128, 512, 768
V = 30522
CORES = 8
VSH = 3840           # vocab columns per core; 8*3840 = 30720 >= 30522
NT = 480             # psum tile width
NVT = VSH // NT      # 8
KC = H // 128        # 6
SC = S // 128        # 4
KU = 128             # padded unique-words per (core, batch)

F32 = mybir.dt.float32
F32R = mybir.dt.float32r
BF16 = mybir.dt.bfloat16
EXP_DT = BF16        # resident exp buffer dtype
NEG_BIG = -1.0e30

AX = mybir.AluOpType
AF = mybir.ActivationFunctionType

CWN = SC * KU        # not used; kept for clarity
A2AW = 264  # AllToAll row width: 128 hi + 128 lo + p + pad

_CACHE = {}


def _gen_kernel(fixwaits=True):
    nc = bass.Bass(target_bir_lowering=False)

    attn_b = nc.dram_tensor("attn_b", [HEADS, T, S], F32, kind="ExternalInput")
    wv_b = nc.dram_tensor("wv_b", [S, H], F32, kind="ExternalInput")
    dec_b = nc.dram_tensor("dec_b", [T, H], F32, kind="ExternalInput")
    cmb = nc.dram_tensor("cmb", [S, CORES, KU], F32, kind="ExternalInput")
    wp = nc.dram_tensor("wp", [T, 2 * H], F32, kind="ExternalInput")
    bp = nc.dram_tensor("bp", [T, 1], F32, kind="ExternalInput")
    dect = nc.dram_tensor("dect", [H, B * T], BF16, kind="ExternalInput")
    wgt = nc.dram_tensor("wgt", [H, VSH], BF16, kind="ExternalInput")
    bg = nc.dram_tensor("bg", [1, VSH], BF16, kind="ExternalInput")
    ones1 = nc.dram_tensor("ones1", [1, T], BF16, kind="ExternalInput")
    u_loc = nc.dram_tensor("u_loc", [B, KU, 1], F32, kind="ExternalInput")
    iotam = nc.dram_tensor("iotam", [128, VSH], F32, kind="ExternalInput")

    outp = nc.dram_tensor("outp", [B, T, VSH], F32, kind="ExternalOutput")

    NG = 4           # AllGather#2 groups
    GB = B // NG     # batches per group

    with tile.TileContext(nc) as tc:
        with (
            tc.tile_pool(name="res", bufs=1) as res,
            tc.tile_pool(name="dram", bufs=1, space="DRAM") as dram,
        ):
            # ---- resident loads (emitted first: feeds the generator) ----
            wgt_sb = res.tile([128, KC * VSH], BF16)
            wgv = wgt_sb[:].rearrange("p (kc v) -> p kc v", kc=KC)
            wgt_in = wgt.rearrange("(kc p) v -> p kc v", p=128)
            HALF = VSH // 2
            nc.sync.dma_start(out=wgv[:, :, 0:HALF], in_=wgt_in[:, :, 0:HALF])
            nc.sync.dma_start(out=wgv[:, :, HALF:VSH], in_=wgt_in[:, :, HALF:VSH])
            dect_sb = res.tile([128, KC * B * T], BF16)
            nc.sync.dma_start(
                out=dect_sb[:].rearrange("p (kc n) -> p kc n", kc=KC),
                in_=dect.rearrange("(kc p) n -> p kc n", p=128),
            )
            dectv = dect_sb[:].rearrange("p (kc b m) -> p kc b m", kc=KC, b=B)
            ones_sb = res.tile([1, T], BF16)
            nc.sync.dma_start(out=ones_sb[:], in_=ones1[:])
            bg_sb = res.tile([1, VSH], BF16)
            nc.sync.dma_start(out=bg_sb[:], in_=bg[:])
            ident = res.tile([128, 128], F32)
            make_identity(nc, ident[:])
            u_sb = res.tile([128, B], F32)
            nc.sync.dma_start(
                out=u_sb[:].rearrange("p (b one) -> p b one", b=B),
                in_=u_loc.rearrange("b p one -> p b one"),
            )
            iot_sb = res.tile([128, VSH], F32)
            nc.sync.dma_start(out=iot_sb[:], in_=iotam[:])

            mparts = res.tile([128, B], F32)
            negm = res.tile([128, B], F32)
            ssump = res.tile([128, B * NVT], F32)
            a_all = res.tile([128, B], F32)
            p_sb = res.tile([128, B], F32)
            cwq_hi = [
                res.tile([128, 128], BF16, tag=f"cwqh{b}", name=f"cwqh{b}")
                for b in range(B)
            ]
            cwq_lo = [
                res.tile([128, 128], BF16, tag=f"cwql{b}", name=f"cwql{b}")
                for b in range(B)
            ]
            diag_a = [
                res.tile([128, 128], BF16, tag=f"diag{b}", name=f"diag{b}")
                for b in range(B)
            ]

            cw_in = dram.tile([CORES * 128, A2AW], F32)
            cw_out = dram.tile([CORES * 128, A2AW], F32)
            warm_in = dram.tile([128, 8], F32)
            warm_out = dram.tile([CORES * 128, 8], F32)
            ms_in = [
                dram.tile([128, 2 * GB], F32, tag=f"msi{g}", name=f"msi{g}")
                for g in range(NG)
            ]
            ms_out = [
                dram.tile([CORES * 128, 2 * GB], F32, tag=f"mso{g}", name=f"mso{g}")
                for g in range(NG)
            ]

            # warm up the collectives path: the first collective pays a large
            # one-time ncfw/TOPSP startup cost; absorb it under phase A/gen.
            warm_sb = res.tile([128, 8], F32)
            nc.gpsimd.memset(warm_sb[:], 0.0)
            nc.gpsimd.dma_start(out=warm_in[:], in_=warm_sb[:])
            nc.gpsimd.collective_compute(
                "AllGather", AX.bypass,
                replica_groups=[list(range(CORES))],
                ins=[warm_in.opt()], outs=[warm_out.opt()],
            )

            # ================= Phase A (own batch only) ===================
            with (
                tc.tile_pool(name="pa", bufs=1) as pa,
                tc.tile_pool(name="pa_ps", bufs=2, space="PSUM") as pa_ps,
            ):
                att = pa.tile([128, HEADS * S], F32)
                nc.gpsimd.dma_start(
                    out=att[:].rearrange("p (h s) -> p h s", h=HEADS),
                    in_=attn_b.rearrange("h t s -> t h s"),
                )
                attv = att[:].rearrange("p (h s) -> p h s", h=HEADS)
                s01 = pa.tile([128, S], F32)
                s23 = pa.tile([128, S], F32)
                s45 = pa.tile([128, S], F32)
                s67 = pa.tile([128, S], F32)
                nc.vector.tensor_tensor(out=s01[:], in0=attv[:, 0], in1=attv[:, 1], op=AX.add)
                nc.vector.tensor_tensor(out=s23[:], in0=attv[:, 2], in1=attv[:, 3], op=AX.add)
                nc.vector.tensor_tensor(out=s45[:], in0=attv[:, 4], in1=attv[:, 5], op=AX.add)
                nc.vector.tensor_tensor(out=s67[:], in0=attv[:, 6], in1=attv[:, 7], op=AX.add)
                nc.vector.tensor_tensor(out=s01[:], in0=s01[:], in1=s23[:], op=AX.add)
                nc.vector.tensor_tensor(out=s45[:], in0=s45[:], in1=s67[:], op=AX.add)
                aw = pa.tile([128, S], F32)
                nc.vector.tensor_tensor(out=aw[:], in0=s01[:], in1=s45[:], op=AX.add)

                awT = pa.tile([128, S], F32)
                for j in range(SC):
                    tp = pa_ps.tile([128, 128], F32, tag="tp")
                    nc.tensor.transpose(
                        out=tp[:], in_=aw[:, j * 128:(j + 1) * 128], identity=ident[:]
                    )
                    nc.vector.tensor_copy(out=awT[:, j * 128:(j + 1) * 128], in_=tp[:])

                wvt = pa.tile([128, SC * H], F32)
                nc.gpsimd.dma_start(
                    out=wvt[:].rearrange("p (j h) -> p j h", j=SC),
                    in_=wv_b.rearrange("(j p) h -> p j h", p=128),
                )
                wvv = wvt[:].rearrange("p (j h) -> p j h", j=SC)
                wp_sb = pa.tile([128, 2 * H], F32)
                nc.gpsimd.dma_start(out=wp_sb[:], in_=wp[:])
                bp_sb = pa.tile([128, 1], F32)
                nc.gpsimd.dma_start(out=bp_sb[:], in_=bp[:])

                scr = pa.tile([128, H], F32)
                rvec = pa.tile([128, SC], F32)
                for j in range(SC):
                    nc.vector.tensor_tensor(
                        out=scr[:], in0=wvv[:, j], in1=wp_sb[:, H:2 * H], op=AX.mult
                    )
                    nc.vector.tensor_reduce(
                        out=rvec[:, j:j + 1], in_=scr[:],
                        axis=mybir.AxisListType.X, op=AX.add,
                    )
                pre2 = pa_ps.tile([128, 1], F32, tag="pre2")
                for j in range(SC):
                    nc.tensor.matmul(
                        out=pre2[:], lhsT=awT[:, j * 128:(j + 1) * 128],
                        rhs=rvec[:, j:j + 1], start=(j == 0), stop=(j == SC - 1),
                    )

                dec_sb = pa.tile([128, H], F32)
                nc.gpsimd.dma_start(out=dec_sb[:], in_=dec_b[:])
                scr2 = pa.tile([128, H], F32)
                pre1 = pa.tile([128, 1], F32)
                nc.vector.tensor_tensor(
                    out=scr2[:], in0=dec_sb[:], in1=wp_sb[:, 0:H], op=AX.mult
                )
                nc.vector.tensor_reduce(
                    out=pre1[:], in_=scr2[:], axis=mybir.AxisListType.X, op=AX.add
                )
                nc.vector.tensor_tensor(out=pre1[:], in0=pre1[:], in1=bp_sb[:], op=AX.add)
                pre = pa.tile([128, 1], F32)
                nc.vector.tensor_tensor(out=pre[:], in0=pre1[:], in1=pre2[:], op=AX.add)
                p_t = pa.tile([128, 1], F32)
                nc.scalar.activation(out=p_t[:], in_=pre[:], func=AF.Sigmoid)
                q_t = pa.tile([128, 1], F32)
                nc.vector.tensor_scalar(
                    out=q_t[:], in0=p_t[:], scalar1=-1.0, scalar2=1.0,
                    op0=AX.mult, op1=AX.add,
                )

                awq = pa.tile([128, S], F32)
                nc.vector.tensor_scalar(
                    out=awq[:], in0=aw[:], scalar1=q_t[:, 0:1], scalar2=None,
                    op0=AX.mult,
                )
                awqT = pa.tile([128, S], F32)
                for j in range(SC):
                    tp = pa_ps.tile([128, 128], F32, tag="tp")
                    nc.tensor.transpose(
                        out=tp[:], in_=awq[:, j * 128:(j + 1) * 128], identity=ident[:]
                    )
                    nc.vector.tensor_copy(out=awqT[:, j * 128:(j + 1) * 128], in_=tp[:])

                cmb_sb = pa.tile([128, SC * CORES * KU], F32)
                nc.gpsimd.dma_start(
                    out=cmb_sb[:].rearrange("p (j c k) -> p j c k", j=SC, c=CORES),
                    in_=cmb.rearrange("(j p) c k -> p j c k", p=128),
                )
                cmbv = cmb_sb[:].rearrange("p (j c k) -> p j c k", j=SC, c=CORES)
                contrib = pa.tile([128, CORES * A2AW], F32)
                cv = contrib[:].rearrange("k (c n) -> k c n", c=CORES)
                hi16 = pa.tile([128, 128], BF16)
                for c in range(CORES):
                    cps = pa_ps.tile([128, 128], F32, tag="cps")
                    for j in range(SC):
                        nc.tensor.matmul(
                            out=cps[:], lhsT=cmbv[:, j, c],
                            rhs=awqT[:, j * 128:(j + 1) * 128],
                            start=(j == 0), stop=(j == SC - 1),
                        )
                    nc.vector.tensor_copy(out=hi16[:], in_=cps[:])
                    nc.vector.tensor_copy(out=cv[:, c, 0:128], in_=hi16[:])
                    nc.vector.tensor_tensor(
                        out=cv[:, c, 128:256], in0=cps[:], in1=hi16[:],
                        op=AX.subtract,
                    )
                    nc.vector.tensor_copy(out=cv[:, c, 256:257], in_=p_t[:])
                    nc.gpsimd.memset(cv[:, c, 257:A2AW], 0.0)
                nc.gpsimd.dma_start(
                    out=cw_in.rearrange("(c k) n -> k c n", k=128), in_=cv[:]
                )

            nc.gpsimd.collective_compute(
                "AllToAll", AX.bypass,
                replica_groups=[list(range(CORES))],
                ins=[cw_in.opt()], outs=[cw_out.opt()],
            )

            with tc.tile_pool(name="pcw", bufs=2) as pcw:
                def cw_readback():
                    for b in range(B):
                        nc.gpsimd.dma_start(
                            out=cwq_hi[b][:],
                            in_=cw_out[b * 128:(b + 1) * 128, 0:128],
                        )
                        nc.gpsimd.dma_start(
                            out=cwq_lo[b][:],
                            in_=cw_out[b * 128:(b + 1) * 128, 128:256],
                        )
                    nc.gpsimd.dma_start(
                        out=p_sb[:].rearrange("p (b one) -> p b one", b=B),
                        in_=cw_out.rearrange("(b p) n -> p b n", p=128)[:, :, 256:257],
                    )

                # ========== interleaved: gen(b) + grouped AG2 + fixup + mix =====
                with (
                    tc.tile_pool(name="pexp", bufs=1) as pexp,
                    tc.tile_pool(name="pfx", bufs=2) as pfx,
                    tc.tile_pool(name="pm", bufs=3) as pm,
                    tc.tile_pool(name="gen_ps", bufs=4, space="PSUM") as gen_ps,
                    tc.tile_pool(name="m_ps", bufs=4, space="PSUM") as m_ps,
                ):
                    exp_t = {}

                    def gen_batch(b):
                        exp_t[b] = pexp.tile(
                            [128, VSH], EXP_DT, tag=f"exp{b}", name=f"exp{b}"
                        )
                        for vt in range(NVT):
                            ps = gen_ps.tile([128, NT], F32, tag="gen", name="gen")
                            for kc in range(KC):
                                nc.tensor.matmul(
                                    out=ps[:], lhsT=dectv[:, kc, b],
                                    rhs=wgv[:, kc, vt * NT:(vt + 1) * NT],
                                    start=(kc == 0), stop=False,
                                )
                            nc.tensor.matmul(
                                out=ps[:], lhsT=ones_sb[0:1, :],
                                rhs=bg_sb[0:1, vt * NT:(vt + 1) * NT],
                                start=False, stop=True,
                            )
                            if vt == 0:
                                nc.vector.tensor_reduce(
                                    out=mparts[:, b:b + 1], in_=ps[:],
                                    axis=mybir.AxisListType.X, op=AX.max,
                                )
                                nc.vector.tensor_scalar(
                                    out=negm[:, b:b + 1], in0=mparts[:, b:b + 1],
                                    scalar1=-1.0, scalar2=None, op0=AX.mult,
                                )
                            nc.scalar.activation(
                                out=exp_t[b][:, vt * NT:(vt + 1) * NT], in_=ps[:],
                                func=AF.Exp, bias=negm[:, b:b + 1], scale=1.0,
                                accum_out=ssump[:, b * NVT + vt:b * NVT + vt + 1],
                            )

                    def stage_ag(g):
                        msc = pfx.tile([128, 2 * GB], F32, tag="msc", name="msc")
                        ssv = ssump[:].rearrange("p (b v) -> p b v", b=B)
                        for i in range(GB):
                            b = g * GB + i
                            nc.vector.tensor_copy(
                                out=msc[:, i:i + 1], in_=mparts[:, b:b + 1]
                            )
                            nc.vector.tensor_reduce(
                                out=msc[:, GB + i:GB + i + 1], in_=ssv[:, b],
                                axis=mybir.AxisListType.X, op=AX.add,
                            )
                        nc.gpsimd.dma_start(out=ms_in[g][:], in_=msc[:])
                        nc.gpsimd.collective_compute(
                            "AllGather", AX.bypass,
                            replica_groups=[list(range(CORES))],
                            ins=[ms_in[g].opt()], outs=[ms_out[g].opt()],
                        )

                    def fixup(g):
                        agg = pfx.tile([128, CORES * 2 * GB], F32, tag="agg", name="agg")
                        nc.gpsimd.dma_start(
                            out=agg[:].rearrange("p (c n) -> p c n", c=CORES),
                            in_=ms_out[g].rearrange("(c p) n -> p c n", p=128),
                        )
                        aggv = agg[:].rearrange("p (c n) -> p c n", c=CORES)
                        for i in range(GB):
                            b = g * GB + i
                            em = pfx.tile([128, CORES], F32, tag="em", name="em")
                            junk = pfx.tile([128, CORES], F32, tag="junk", name="junk")
                            bigm = pfx.tile([128, 1], F32, tag="bigm", name="bigm")
                            nbigm = pfx.tile([128, 1], F32, tag="nbigm", name="nbigm")
                            ssum = pfx.tile([128, 1], F32, tag="ssum", name="ssum")
                            sinv = pfx.tile([128, 1], F32, tag="sinv", name="sinv")
                            eo = pfx.tile([128, 1], F32, tag="eo", name="eo")
                            nc.vector.tensor_reduce(
                                out=bigm[:], in_=aggv[:, :, i],
                                axis=mybir.AxisListType.X, op=AX.max,
                            )
                            nc.vector.tensor_scalar(
                                out=nbigm[:], in0=bigm[:], scalar1=-1.0,
                                scalar2=None, op0=AX.mult,
                            )
                            nc.scalar.activation(
                                out=em[:], in_=aggv[:, :, i], func=AF.Exp,
                                bias=nbigm[:, 0:1], scale=1.0,
                            )
                            nc.vector.tensor_tensor(
                                out=junk[:], in0=em[:], in1=aggv[:, :, GB + i],
                                op=AX.mult,
                            )
                            nc.vector.tensor_reduce(
                                out=ssum[:], in_=junk[:],
                                axis=mybir.AxisListType.X, op=AX.add,
                            )
                            nc.vector.reciprocal(out=sinv[:], in_=ssum[:])
                            nc.vector.tensor_tensor(
                                out=eo[:], in0=mparts[:, b:b + 1], in1=bigm[:],
                                op=AX.subtract,
                            )
                            nc.scalar.activation(out=eo[:], in_=eo[:], func=AF.Exp)
                            nc.vector.tensor_tensor(
                                out=eo[:], in0=eo[:], in1=sinv[:], op=AX.mult
                            )
                            nc.vector.tensor_tensor(
                                out=a_all[:, b:b + 1], in0=eo[:], in1=p_sb[:, b:b + 1],
                                op=AX.mult,
                            )
                            nc.vector.tensor_tensor(
                                out=diag_a[b][:], in0=ident[:],
                                in1=a_all[:, b:b + 1].to_broadcast([128, 128]),
                                op=AX.mult,
                            )

                    def mix(g):
                        for i in range(GB):
                            b = g * GB + i
                            for vt in range(NVT):
                                oh = pm.tile([128, NT], BF16, tag="oh", name="oh")
                                nc.vector.tensor_tensor(
                                    out=oh[:], in0=iot_sb[:, vt * NT:(vt + 1) * NT],
                                    in1=u_sb[:, b:b + 1].to_broadcast([128, NT]),
                                    op=AX.is_equal,
                                )
                                av = m_ps.tile([128, NT], F32, tag="av", name="av")
                                nc.tensor.matmul(
                                    out=av[:], lhsT=cwq_hi[b][:], rhs=oh[:],
                                    start=True, stop=False,
                                )
                                nc.tensor.matmul(
                                    out=av[:], lhsT=cwq_lo[b][:], rhs=oh[:],
                                    start=False, stop=False,
                                )
                                nc.tensor.matmul(
                                    out=av[:], lhsT=diag_a[b][:],
                                    rhs=exp_t[b][:, vt * NT:(vt + 1) * NT],
                                    start=False, stop=True,
                                )
                                ot = pm.tile([128, NT], F32, tag="ot", name="ot")
                                nc.scalar.copy(out=ot[:], in_=av[:])
                                nc.sync.dma_start(
                                    out=outp[b, :, vt * NT:(vt + 1) * NT], in_=ot[:]
                                )

                    for b in range(B):
                        gen_batch(b)
                        if b == 1:
                            cw_readback()
                        if b % 2 == 1:
                            stage_ag(b // 2)
                        if b >= 3 and b % 2 == 1:
                            g = (b - 3) // 2
                            fixup(g)
                            mix(g)
                    fixup(NG - 1)
                    mix(NG - 1)

    if fixwaits:
        _fix_multiwait(nc)
    return nc


# ---------------------------------------------------------------------------
# Host-side driver.
# ---------------------------------------------------------------------------


def _prep_inputs(decoder_outputs, attn_dist, word_vec, words, W_gen, b_gen, W_p, b_p):
    f32 = np.float32
    dec = np.asarray(decoder_outputs, f32)
    attn = np.asarray(attn_dist, f32)
    wv = np.asarray(word_vec, f32)
    words = np.asarray(words)
    W_gen = np.asarray(W_gen, f32)
    b_gen = np.asarray(b_gen, f32)
    W_p = np.asarray(W_p, f32)
    b_p = np.asarray(b_p, f32)

    WT = np.ascontiguousarray(W_gen.T)                      # [H, V]
    wgt_pad = np.zeros((H, CORES * VSH), BF16_NP)
    wgt_pad[:, :V] = WT.astype(BF16_NP)
    bg_pad = np.full((CORES * VSH,), NEG_BIG, BF16_NP)
    bg_pad[:V] = b_gen.astype(BF16_NP)
    # [H, B*T] contiguous, bf16
    dect = np.ascontiguousarray(
        dec.transpose(2, 0, 1).reshape(H, B * T).astype(BF16_NP)
    )

    wp_scaled = np.tile(W_p.reshape(1, 2 * H), (T, 1)).astype(f32)
    wp_scaled[:, H:] *= 1.0 / HEADS                          # fold head-mean
    bp_b = np.full((T, 1), float(b_p.reshape(-1)[0]), f32)
    ones_row = np.ones((1, T), BF16_NP)

    # per (core, batch) dedup: local indices + combining matrix
    u_all = np.full((CORES, B, KU, 1), -1.0, f32)
    cmb_all = np.zeros((B, S, CORES, KU), f32)
    for b in range(B):
        w = np.asarray(words[b], np.int64)
        for c in range(CORES):
            lo, hi = c * VSH, (c + 1) * VSH
            mask = (w >= lo) & (w < hi)
            uniq = np.unique(w[mask])
            k = len(uniq)
            assert k <= KU, f"unique words {k} exceeds KU={KU}"
            u_all[c, b, :k, 0] = (uniq - lo).astype(f32)
            if k:
                pos = np.searchsorted(uniq, w[mask])
                cmb_all[b, np.nonzero(mask)[0], c, pos] = 1.0 / HEADS

    iotam = np.tile(np.arange(VSH, dtype=f32)[None, :], (128, 1))
    in_maps = []
    for c in range(CORES):
        in_maps.append({
            "attn_b": np.ascontiguousarray(attn[c]),
            "wv_b": np.ascontiguousarray(wv[c]),
            "dec_b": np.ascontiguousarray(dec[c]),
            "cmb": np.ascontiguousarray(cmb_all[c]),
            "wp": wp_scaled,
            "bp": bp_b,
            "dect": dect,
            "wgt": np.ascontiguousarray(wgt_pad[:, c * VSH:(c + 1) * VSH]),
            "bg": np.ascontiguousarray(bg_pad[c * VSH:(c + 1) * VSH]).reshape(1, VSH),
            "ones1": ones_row,
            "u_loc": np.ascontiguousarray(u_all[c]),
            "iotam": iotam,
        })
    return in_maps


LAST_RESULTS = None


def kernel(decoder_outputs, attn_dist, word_vec, words, W_gen, b_gen, W_p, b_p):
    global LAST_RESULTS
    in_maps = _prep_inputs(
        decoder_outputs, attn_dist, word_vec, words, W_gen, b_gen, W_p, b_p
    )
    if "nc" not in _CACHE:
        _CACHE["nc"] = _gen_kernel()
    nc = _CACHE["nc"]
    res = run_bass_kernel_spmd(nc, in_maps, core_ids=list(range(CORES)))
    LAST_RESULTS = res
    out = np.empty((B, T, V), np.float32)
    for c in range(CORES):
        lo = c * VSH
        hi = min(V, lo + VSH)
        out[:, :, lo:hi] = res.results[c]["outp"][:, :, : hi - lo]
    return out
